# revision 8
# baseline (speedup 1.0000x reference)
"""Trainium2 Bass kernel for nn_MatchNet (MatchNet forward, sparse_attention).

Computes, for PROCESS_STEPS=4 (reference semantics):
    h_r = 0; c = 0
    repeat 4x:
        gates = qm @ W_ih.T + h_r @ W_hh.T + b          # [B, 8192]
        i,f,g,o = split(gates); c = sig(f)*c + sig(i)*tanh(g)
        h_new = sig(o)*tanh(c)
        h = qm + h_new[:, :D]
        attn = softmax(h @ sm.T); r = attn @ sm
        h_r = [h, r]
    return h @ sm.T                                      # [B, 512]

Algebraic restructuring used here (exact, not approximations):
  * Only h_new[:, :D] is ever consumed, and the LSTM math is elementwise,
    so only gate columns {0:D, 2D:3D, 4D:5D, 6D:7D} (4096 of 8192) are
    needed -> all gate matmuls are halved.
  * gates_sel(t) = G1b + Whh_h @ h(t-1) + SW @ attn(t-1), where
    G1b = Wih_sel @ qm.T + b_sel (step-invariant, computed once),
    SW  = sm @ Whh_r_sel.T       (step-invariant, computed once),
    because h_r = [h, attn @ sm].  r is never materialized.
  * Step 1 has h_r = 0 -> gates(1) = G1b.  Step 4 needs no softmax.

Sharding: data-parallel over the batch (2048 -> 8 x 256), weights and
support replicated; no collectives.  Each core computes scores[256, 512];
host concatenates.

Layout per core (gate-transposed): gates/h/c live as [dim, batch] with the
hidden dim on partitions, so weights are PE-stationary and activations are
the moving operand.  Softmax runs in [batch, S] layout (free-dim reductions
+ per-partition scalars), and attn is transposed back via PE transposes.
Numerics: bf16 for the big gate matmuls, float32r (TF32-class) for the
output logits matmul, fp32 elementwise state.
"""

import sys

for _p in ("/opt/trn_rl_repo", "/opt/pypackages"):
    if _p not in sys.path:
        sys.path.insert(0, _p)

import numpy as np
import ml_dtypes

import concourse.bacc as bacc
import concourse.mybir as mybir
from concourse import tile
from concourse import bass_utils

F32 = mybir.dt.float32
F32R = mybir.dt.float32r
BF16 = mybir.dt.bfloat16
AF = mybir.ActivationFunctionType
ALU = mybir.AluOpType
AX = mybir.AxisListType

P = 128
D = 1024            # input dim
S = 512             # support size
B = 2048            # total batch
NCORES = 8
BL = B // NCORES    # 256 batch rows per core
G = 4 * D           # 4096 selected gate columns
KD = D // P         # 8 contraction chunks over D
KS = S // P         # 4 contraction chunks over S
NU = KD             # 8 unit-chunks (first-D hidden units)
NGT = 4             # gate types i,f,g,o
NB = BL // P        # 2 batch chunks
STEPS = 4

_PROG_CACHE = {}


def _emit(nc):
    qmT_d = nc.dram_tensor("qmT", (D, BL), F32R, kind="ExternalInput").ap()
    wih_d = nc.dram_tensor("wih", (NU, KD, P, NGT * P), F32R, kind="ExternalInput").ap()
    whhh_d = nc.dram_tensor("whhh", (NU, KD, P, NGT * P), BF16, kind="ExternalInput").ap()
    whhr_d = nc.dram_tensor("whhr", (KD, KD, P, 512), F32R, kind="ExternalInput").ap()
    smT_d = nc.dram_tensor("smT", (D, S), F32, kind="ExternalInput").ap()
    smTr_d = nc.dram_tensor("smTr", (D, S), F32R, kind="ExternalInput").ap()
    b_d = nc.dram_tensor("bsel", (G,), F32, kind="ExternalInput").ap()
    identf_d = nc.dram_tensor("identf", (P, P), F32, kind="ExternalInput").ap()
    scores_d = nc.dram_tensor("scores", (BL, S), F32, kind="ExternalOutput").ap()

    with tile.TileContext(nc) as tc:
        with tc.tile_pool(name="cw", bufs=1) as cw, \
             tc.tile_pool(name="st", bufs=1) as st, \
             tc.tile_pool(name="wk", bufs=2) as wk, \
             tc.tile_pool(name="ws", bufs=3) as ws, \
             tc.tile_pool(name="ps", bufs=8, space="PSUM") as ps:

            # ---- resident tensors -------------------------------------
            qmT_sb = cw.tile([P, KD * BL], F32R, name="qmT_sb")
            smT_sb = cw.tile([P, KD * S], F32, name="smT_sb")
            smTr_sb = cw.tile([P, KD * S], F32R, name="smTr_sb")
            sw_sb = cw.tile([P, KS * G], F32R, name="sw_sb")
            g1b_sb = cw.tile([P, (NGT * NU) * BL], F32, name="g1b_sb")
            b_sb = cw.tile([P, NGT * NU], F32, name="b_sb")
            ident_f32_sb = cw.tile([P, P], F32, name="ident_f32_sb")

            hT_sb = st.tile([P, KD * BL], F32, name="hT_sb")
            hbf_sb = [st.tile([P, KD * BL], BF16, name=f"hbf{i}_sb") for i in range(2)]
            cT_sb = st.tile([P, KD * BL], F32, name="cT_sb")
            attnT_sb = [st.tile([P, KS * BL], F32R, name=f"attnT{i}_sb") for i in range(2)]
            exp_sb = st.tile([P, NB * S], F32, name="exp_sb")

            # ---- small loads ------------------------------------------
            nc.sync.dma_start(b_sb[:, 0:NGT * NU],
                              b_d.rearrange("(m p) -> p m", p=P))
            nc.sync.dma_start(ident_f32_sb[:], identf_d)
            for k in range(KD):
                nc.sync.dma_start(qmT_sb[:, k * BL:(k + 1) * BL],
                                  qmT_d[k * P:(k + 1) * P, :])
            for k in range(KD):
                nc.sync.dma_start(smT_sb[:, k * S:(k + 1) * S],
                                  smT_d[k * P:(k + 1) * P, :])
            for k in range(KD):
                nc.sync.dma_start(smTr_sb[:, k * S:(k + 1) * S],
                                  smTr_d[k * P:(k + 1) * P, :])

            def bslice(t, idx, width):
                return t[:, idx * width:(idx + 1) * width]

            # ---- elementwise LSTM tail for one unit-chunk u -----------
            def lstm_tail(u, gi, gf, gg, go, step1, hbf_cur):
                """gi/gf/gg/go: [P, BL] gate pre-activations (f32 or bf16 APs).
                Updates cT/hT/hbf chunk u."""
                cchunk = bslice(cT_sb, u, BL)
                si = wk.tile([P, BL], F32, name=f"si_{u}", tag="si")
                tg = wk.tile([P, BL], F32, name=f"tg_{u}", tag="tg")
                so = wk.tile([P, BL], F32, name=f"so_{u}", tag="so")
                nc.scalar.activation(si[:], gi, AF.Sigmoid)
                nc.scalar.activation(tg[:], gg, AF.Tanh)
                nc.scalar.activation(so[:], go, AF.Sigmoid)
                if step1:
                    # c = sig(i) * tanh(g)
                    nc.vector.tensor_mul(cchunk, si[:], tg[:])
                else:
                    sf = wk.tile([P, BL], F32, name=f"sf_{u}", tag="sf")
                    nc.scalar.activation(sf[:], gf, AF.Sigmoid)
                    nc.vector.tensor_mul(si[:], si[:], tg[:])      # si = sig(i)*tanh(g)
                    nc.vector.tensor_mul(sf[:], sf[:], cchunk)     # sf = sig(f)*c
                    nc.vector.tensor_add(cchunk, si[:], sf[:])
                tc_ = wk.tile([P, BL], F32, name=f"tc_{u}", tag="tc")
                nc.scalar.activation(tc_[:], cchunk, AF.Tanh)
                nc.vector.tensor_mul(so[:], so[:], tc_[:])         # so = sig(o)*tanh(c)
                qchunk = bslice(qmT_sb, u, BL).bitcast(F32)
                nc.vector.tensor_add(bslice(hT_sb, u, BL), so[:], qchunk)
                nc.vector.tensor_add(bslice(hbf_cur, u, BL), so[:], qchunk)

            # ---- phase A: G1 = Wih_sel @ qm.T (+bias), fused step-1 ----
            for u in range(NU):
                pg = [ps.tile([P, BL], F32, name=f"pg1_{u}_{gt}", tag="ps")
                      for gt in range(NGT)]
                for k in range(KD):
                    wt = ws.tile([P, NGT * P], F32R, name=f"wih_{u}_{k}", tag="wih")
                    nc.sync.dma_start(wt[:], wih_d[u, k])
                    for gt in range(NGT):
                        nc.tensor.matmul(pg[gt][:], wt[:, gt * P:(gt + 1) * P],
                                         bslice(qmT_sb, k, BL),
                                         start=(k == 0), stop=(k == KD - 1))
                gb = []
                for gt in range(NGT):
                    m = gt * NU + u
                    nc.scalar.activation(bslice(g1b_sb, m, BL), pg[gt][:],
                                         AF.Identity, bias=b_sb[:, m:m + 1])
                    gb.append(bslice(g1b_sb, m, BL))
                lstm_tail(u, gb[0], gb[1], gb[2], gb[3], step1=True,
                          hbf_cur=hbf_sb[1])

            # ---- logits / softmax / attn-transpose helpers ------------
            def logits(step):
                pls = []
                for bc in range(NB):
                    pl = ps.tile([P, S], F32, name=f"pl_{step}_{bc}", tag="ps")
                    for k in range(KD):
                        nc.tensor.matmul(
                            pl[:],
                            hT_sb[:, k * BL + bc * P: k * BL + (bc + 1) * P],
                            bslice(smT_sb, k, S),
                            start=(k == 0), stop=(k == KD - 1))
                    pls.append(pl)
                return pls

            def softmax_transpose(step, pls, attnT_cur):
                for bc in range(NB):
                    nm = wk.tile([P, 1], F32, name=f"nm_{step}_{bc}", tag="nm")
                    sm_sum = wk.tile([P, 1], F32, name=f"ssum_{step}_{bc}", tag="ssum")
                    inv = wk.tile([P, 1], F32, name=f"inv_{step}_{bc}", tag="inv")
                    nc.vector.tensor_reduce(nm[:], pls[bc][:], axis=AX.X,
                                            op=ALU.max, negate=True)
                    nc.scalar.activation(bslice(exp_sb, bc, S), pls[bc][:], AF.Exp,
                                         bias=nm[:], accum_out=sm_sum[:])
                    nc.vector.reciprocal(inv[:], sm_sum[:])
                    nc.vector.tensor_scalar_mul(bslice(exp_sb, bc, S),
                                                bslice(exp_sb, bc, S), inv[:])
                for bc in range(NB):
                    for s in range(KS):
                        pt = ps.tile([P, P], F32, name=f"pt_{step}_{bc}_{s}", tag="ps")
                        nc.tensor.transpose(
                            pt[:], exp_sb[:, bc * S + s * P: bc * S + (s + 1) * P],
                            ident_f32_sb[:])
                        nc.vector.tensor_copy(
                            attnT_cur[:, s * BL + bc * P: s * BL + (bc + 1) * P],
                            pt[:])

            # step-1 logits/softmax
            pls = logits(1)
            softmax_transpose(1, pls, attnT_sb[1])

            # ---- phase B: SW = sm @ Whh_r_sel.T  [S, G] ---------------
            for n in range(KD):          # 8 chunks of 512 gate cols
                psw = [ps.tile([P, 512], F32, name=f"psw_{n}_{s}", tag="ps")
                       for s in range(KS)]
                for k in range(KD):
                    wt = ws.tile([P, 512], F32R, name=f"whhr_{n}_{k}", tag="whhr")
                    nc.sync.dma_start(wt[:], whhr_d[n, k])
                    for s in range(KS):
                        nc.tensor.matmul(psw[s][:],
                                         smTr_sb[:, k * S + s * P: k * S + (s + 1) * P],
                                         wt[:],
                                         start=(k == 0), stop=(k == KD - 1))
                for s in range(KS):
                    nc.vector.tensor_copy(
                        sw_sb[:, s * G + n * 512: s * G + (n + 1) * 512], psw[s][:])

            # ---- steps 2..4 -------------------------------------------
            for step in range(2, STEPS + 1):
                hbf_prev = hbf_sb[(step - 1) % 2]
                hbf_cur = hbf_sb[step % 2]
                aT_prev = attnT_sb[(step - 1) % 2]
                aT_cur = attnT_sb[step % 2]
                for u in range(NU):
                    gsb = []
                    pgs = [ps.tile([P, BL], F32, name=f"pg{step}_{u}_{gt}", tag="ps")
                           for gt in range(NGT)]
                    for k in range(KD):
                        wt = ws.tile([P, NGT * P], BF16,
                                     name=f"whhh_{step}_{u}_{k}", tag="whhh")
                        nc.sync.dma_start(wt[:], whhh_d[u, k])
                        for gt in range(NGT):
                            nc.tensor.matmul(
                                pgs[gt][:], wt[:, gt * P:(gt + 1) * P],
                                bslice(hbf_prev, k, BL),
                                start=(k == 0), stop=False)
                    for gt in range(NGT):
                        m = gt * NU + u
                        pgt = pgs[gt]
                        for s in range(KS):
                            nc.tensor.matmul(
                                pgt[:],
                                sw_sb[:, s * G + m * P: s * G + (m + 1) * P],
                                bslice(aT_prev, s, BL),
                                start=False, stop=(s == KS - 1))
                        gt_sb = wk.tile([P, BL], F32, name=f"g{step}_{u}_{gt}",
                                        tag="gates", bufs=6)
                        nc.vector.scalar_tensor_tensor(
                            gt_sb[:], pgt[:], 1.0, bslice(g1b_sb, m, BL),
                            op0=ALU.mult, op1=ALU.add)
                        gsb.append(gt_sb)
                    lstm_tail(u, gsb[0][:], gsb[1][:], gsb[2][:], gsb[3][:],
                              step1=False, hbf_cur=hbf_cur)
                pls = logits(step)
                if step < STEPS:
                    softmax_transpose(step, pls, aT_cur)
                else:
                    for bc in range(NB):
                        # exp_sb is dead in step 4; reuse it as output staging
                        nc.vector.tensor_copy(bslice(exp_sb, bc, S), pls[bc][:])
                        nc.sync.dma_start(scores_d[bc * P:(bc + 1) * P, :],
                                          bslice(exp_sb, bc, S))


def _build_program():
    if "nc" in _PROG_CACHE:
        return _PROG_CACHE["nc"]
    nc = bacc.Bacc("TRN2", target_bir_lowering=False, debug=False,
                   num_devices=NCORES)
    _emit(nc)
    nc.compile()
    _PROG_CACHE["nc"] = nc
    return nc


def _host_prep(support_mean, query_mean, W_ih, W_hh, b_ih, b_hh):
    """Build the per-core input maps (layout transforms only)."""
    sel = np.concatenate([np.arange(2 * k * D, (2 * k + 1) * D) for k in range(4)])
    wih_sel = np.ascontiguousarray(W_ih[sel])              # [G, D]
    whh_sel = np.ascontiguousarray(W_hh[sel])              # [G, 2D]
    b_sel = (b_ih + b_hh)[sel].astype(np.float32)          # [G]

    bf = ml_dtypes.bfloat16

    def tile_ukp(wT, dt):
        # [u, k, p, gt*128+j] = wT[k*128+p, gt*1024 + u*128 + j]
        return np.ascontiguousarray(
            wT.reshape(KD, P, NGT, NU, P).transpose(3, 0, 1, 2, 4)
              .reshape(NU, KD, P, NGT * P).astype(dt))

    wihT = np.ascontiguousarray(wih_sel.T)                      # [D, G] f32
    wih_td = tile_ukp(wihT, np.float32)
    whhhT = np.ascontiguousarray(whh_sel[:, :D].T)              # [D, G]
    whhh_td = tile_ukp(whhhT, bf)
    whhrT = np.ascontiguousarray(whh_sel[:, D:].T)              # [D, G] f32
    whhr_td = np.ascontiguousarray(
        whhrT.reshape(KD, P, KD, 512).transpose(2, 0, 1, 3))    # [n, k, p, 512]

    smT = np.ascontiguousarray(support_mean.T).astype(np.float32)  # [D, S]

    shared = {
        "wih": wih_td, "whhh": whhh_td, "whhr": whhr_td,
        "smT": smT, "smTr": smT, "bsel": b_sel,
        "identf": np.eye(P, dtype=np.float32),
    }
    in_maps = []
    for c in range(NCORES):
        qmT = np.ascontiguousarray(
            query_mean[c * BL:(c + 1) * BL].T).astype(np.float32)  # [D, BL]
        m = dict(shared)
        m["qmT"] = qmT
        in_maps.append(m)
    return in_maps


def run_on_hw(in_maps, **kwargs):
    nc = _build_program()
    last_exc = None
    for _attempt in range(3):
        try:
            return bass_utils.run_bass_kernel_spmd(
                nc, in_maps, core_ids=list(range(NCORES)), **kwargs)
        except Exception as e:  # transient NRT/axon flakes
            last_exc = e
    raise last_exc


def kernel(support_mean, support_var, query_mean, query_var,
           W_ih, W_hh, b_ih, b_hh, **_unused):
    support_mean = np.asarray(support_mean, dtype=np.float32)
    query_mean = np.asarray(query_mean, dtype=np.float32)
    W_ih = np.asarray(W_ih, dtype=np.float32)
    W_hh = np.asarray(W_hh, dtype=np.float32)
    b_ih = np.asarray(b_ih, dtype=np.float32)
    b_hh = np.asarray(b_hh, dtype=np.float32)

    in_maps = _host_prep(support_mean, query_mean, W_ih, W_hh, b_ih, b_hh)
    res = run_on_hw(in_maps)
    return np.concatenate([res.results[c]["scores"] for c in range(NCORES)],
                          axis=0)
